# revision 6
# baseline (speedup 1.0000x reference)
"""Trainium2 Bass kernel for nn_ContractExpand (segment_reduce, 5 scales).

out[n, b, l, e] = relu(segsum_r(x)[b, g(l), :] @ (W[n]/r).T + b[n]/r)  broadcast over groups

Strategy (data-parallel over B across 8 cores, 8 batches each):
 - host: append a ones-column to x ([B,800,301], cast bf16); fold bias + the 1/r
   scale into augmented weights WT_aug[n] = [W[n].T/r ; b[n]/r^2]  (the
   ones-column yields a "count row" equal to r via the segment matmul, so
   r * b/r^2 = b/r).  All matmul operands are bf16 (fp32 matmul streams at
   ~4 cyc/col and needs 2 LDWEIGHTS per MM); PSUM accumulation stays f32.
 - device, per batch:
     1. segsum-matmul: one PE pass computes seg_augT[d, g] for ALL 5 scales at
        once (including the r=1 transpose) into a tile-major PSUM tile
        [d, 8 l-tiles x 192]: stationary = natural x tile [100l x d], moving =
        constant 0/1 S_pack [100, 189] -> ONE matmul per l-tile (+ bank splits).
        100-row l-tiles make every scale's group boundaries tile-aligned.
     2. evacuate PSUM -> SBUF bf16 per scale (de-scrambles tile-major to
        scale-major), on ACT/DVE.
     3. main matmul per scale/g-tile: psum[g<=128, 300] = seg_augT.T @ WT_aug,
        3 K-tile accumulation; ReLU evacuates to SBUF f32 on ACT/DVE.
     4. store with the r-fold row replication done by the DMA itself
        (step-0 broadcast source AP -> fully contiguous DRAM writes).
"""

import numpy as np
import ml_dtypes

import concourse.bass as bass
import concourse.tile as tile
from concourse import bacc, mybir
from concourse.bass_utils import run_bass_kernel_spmd

F32 = mybir.dt.float32
BF16 = mybir.dt.bfloat16

R_SCALES = (1, 2, 4, 10, 25)
B, L, D = 64, 800, 300
NCORES = 8
B_LOC = B // NCORES          # 8 batches per core
LT = 100                     # l-tile rows; all scale group sizes align
NT = L // LT                 # 8 l-tiles
SCOLS = [LT // r for r in R_SCALES]                 # 100 50 25 10 4
SCOFF = np.cumsum([0] + SCOLS).tolist()             # s_pack col offsets
SC = SCOFF[-1]                                      # 189
SCP = 190                                           # padded (even) s_pack cols
G = [L // r for r in R_SCALES]                      # 800 400 200 80 32
POFF = np.cumsum([0] + G).tolist()                  # packed seg col offsets
GTOT = POFF[-1]                                     # 1512
DSLICES = [(0, 128), (128, 128), (256, 45)]         # x_aug col K-tiles (45 incl ones)
BANK = 512                                          # psum bank, f32 elems
PBLK = 192                                          # psum cols per l-tile block (8*192 = 3 banks)


def _mm_sched():
    """Segment-matmul schedule per d-tile: (t, s0, w, dst, start, stop).

    One matmul per l-tile covering all 5 scales (s_pack block [100, 189]),
    split where the 192-wide psum block crosses a 512-col bank boundary.
    start/stop go to the first/last matmul into each bank (zero-region
    discipline).
    """
    mms = []
    for t in range(NT):
        w, s0 = SC, 0
        dst = PBLK * t
        while w > 0:
            w1 = min(w, (dst // BANK + 1) * BANK - dst)
            mms.append([t, s0, w1, dst])
            s0 += w1
            dst += w1
            w -= w1
    first, last = {}, {}
    for i, (t, s0, w, dst) in enumerate(mms):
        bk = dst // BANK
        first.setdefault(bk, i)
        last[bk] = i
    return [
        (t, s0, w, dst, i == first[dst // BANK], i == last[dst // BANK])
        for i, (t, s0, w, dst) in enumerate(mms)
    ]


MM_SCHED = _mm_sched()


def build_s_pack():
    s = np.zeros((LT, NT, SCP), np.float32)
    for t in range(NT):
        for si, r in enumerate(R_SCALES):
            for p in range(LT):
                s[p, t, SCOFF[si] + p // r] = 1.0
    return s.astype(ml_dtypes.bfloat16)


def build_wt_aug(W, b):
    out = np.zeros((5, D + 1, D), np.float64)
    for n, r in enumerate(R_SCALES):
        out[n, :D, :] = np.asarray(W[n], np.float64).T / r
        out[n, D, :] = np.asarray(b[n], np.float64) / (r * r)
    return out.astype(ml_dtypes.bfloat16)


def _body(tc, out_ap, x_ap, wt_ap, spk_ap):
    nc = tc.nc
    with (
        tc.tile_pool(name="consts", bufs=1) as consts,
        tc.tile_pool(name="xp", bufs=2) as xp,
        tc.tile_pool(name="segp", bufs=2) as segp,
        tc.tile_pool(name="yp", bufs=2) as yp,
        tc.tile_pool(name="psp", bufs=1, space="PSUM") as psp,
        tc.tile_pool(name="mpsp", bufs=4, space="PSUM") as mpsp,
    ):
        spk_sb = consts.tile([LT, NT, SCP], BF16, name="spk_sb")
        nc.gpsimd.dma_start(out=spk_sb[:, :, :], in_=spk_ap[:, :, :])
        wtiles = []
        for n in range(5):
            row = []
            for k, (d0, dw) in enumerate(DSLICES):
                w = consts.tile([dw, D], BF16, name=f"w_{n}_{k}")
                nc.gpsimd.dma_start(out=w[:, :], in_=wt_ap[n, d0 : d0 + dw, :])
                row.append(w)
            wtiles.append(row)

        def load_x(b):
            x_sb = xp.tile([LT, NT, 304], BF16, name="x_sb", tag="x")
            nc.gpsimd.dma_start(
                out=x_sb[:, :, 0 : D + 1],
                in_=x_ap[b].rearrange("(t p) d -> p t d", p=LT),
            )
            return x_sb

        def segsum_k(x_sb, k):
            d0, dw = DSLICES[k]
            ps = psp.tile([128, 8 * PBLK], F32, name="segps", tag="segps")
            for t, s0, w, dst, start, stop in MM_SCHED:
                nc.tensor.matmul(
                    ps[0:dw, dst : dst + w],
                    x_sb[:, t, d0 : d0 + dw],
                    spk_sb[:, t, s0 : s0 + w],
                    start=start,
                    stop=stop,
                )
            # de-scramble tile-major psum -> scale-major bf16 seg tile
            seg = segp.tile([dw, GTOT], BF16, name=f"seg{k}", tag=f"seg{k}")
            pst = ps[0:dw, :].rearrange("p (t c) -> p t c", c=PBLK)
            for si in range(5):
                w_ = SCOLS[si]
                src = pst[:, :, SCOFF[si] : SCOFF[si] + w_]
                dst_ = seg[:, POFF[si] : POFF[si] + NT * w_].rearrange(
                    "p (t c) -> p t c", t=NT
                )
                if si == 0:
                    nc.scalar.copy(dst_, src)
                else:
                    nc.vector.tensor_copy(dst_, src)
            return seg

        def main_units(b, segs):
            """Yield 15 closures (one per g-tile matmul+relu, with the scale's
            stores attached to its last unit) for interleaved emission."""
            cnt = 0
            for n, r in enumerate(R_SCALES):
                njf, tail = divmod(G[n], 128)
                nj = njf + (1 if tail else 0)
                y = yp.tile([128, nj, D], F32, name=f"y{n}", tag=f"y{n}")
                for j in range(nj):
                    gw = 128 if j < njf else tail

                    def unit(n=n, r=r, j=j, gw=gw, njf=njf, tail=tail, nj=nj, y=y,
                             cnt=cnt, last=(j == nj - 1)):
                        c0 = POFF[n] + 128 * j
                        mp = mpsp.tile([128, BANK], F32, name="mainps", tag="mainps")
                        for k, (d0, dw) in enumerate(DSLICES):
                            nc.tensor.matmul(
                                mp[0:gw, 0:D],
                                segs[k][0:dw, c0 : c0 + gw],
                                wtiles[n][k][:, :],
                                start=(k == 0),
                                stop=(k == 2),
                            )
                        if cnt % 2 == 0:
                            nc.vector.tensor_scalar_max(
                                y[0:gw, j, :], mp[0:gw, 0:D], 0.0
                            )
                        else:
                            nc.scalar.activation(
                                y[0:gw, j, :],
                                mp[0:gw, 0:D],
                                mybir.ActivationFunctionType.Relu,
                            )
                        if last:
                            emit_stores(n, r, njf, tail, nj, y, b)

                    yield unit
                    cnt += 1

        def emit_stores(n, r, njf, tail, nj, y, b):
            # alternate store issue between the SP (sync) and GpSimd DMA
            # queues so descriptor generation isn't serialized on one ring
            dst = out_ap[n, b]
            engs = [nc.sync, nc.gpsimd]
            if r == 1:
                if njf:
                    engs[b % 2].dma_start(
                        out=dst[0 : njf * 128].rearrange("(j p) e -> p j e", p=128),
                        in_=y[:, 0:njf, :],
                    )
                if tail:
                    engs[(b + 1) % 2].dma_start(
                        out=dst[njf * 128 :], in_=y[0:tail, njf, :]
                    )
            else:
                # DMA APs are capped at 3 dims -> one store per g-tile,
                # row-replication via a step-0 broadcast dim on the source.
                for j in range(nj):
                    gw = 128 if j < njf else tail
                    engs[(b + j) % 2].dma_start(
                        out=dst[j * 128 * r : (j * 128 + gw) * r].rearrange(
                            "(p q) e -> p q e", q=r
                        ),
                        in_=y[0:gw, j, :].unsqueeze(1).to_broadcast((gw, r, D)),
                    )

        # software pipeline: segsum of batch b interleaved with main of b-1,
        # 5 main g-tile units after each segsum k-tile (covers the segps
        # bufs=1 evacuation wait with PE work)
        prev_units = None
        x_cur = load_x(0)
        for b in range(B_LOC):
            x_next = load_x(b + 1) if b + 1 < B_LOC else None
            segs = []
            for k in range(3):
                segs.append(segsum_k(x_cur, k))
                if prev_units is not None:
                    for _ in range(5):
                        u = next(prev_units, None)
                        if u is not None:
                            u()
            prev_units = main_units(b, segs)
            x_cur = x_next
        for u in prev_units:
            u()


def build_module():
    nc = bacc.Bacc("TRN2", target_bir_lowering=False, debug=False)
    x = nc.dram_tensor("x", [B_LOC, L, D + 1], BF16, kind="ExternalInput")
    wt = nc.dram_tensor("wt", [5, D + 1, D], BF16, kind="ExternalInput")
    spk = nc.dram_tensor("spk", [LT, NT, SCP], BF16, kind="ExternalInput")
    out = nc.dram_tensor("out", [5, B_LOC, L, D], F32, kind="ExternalOutput")
    with tile.TileContext(nc) as tc:
        _body(tc, out.ap(), x.ap(), wt.ap(), spk.ap())
    nc.compile()
    return nc


_MODULE = None


def _get_module():
    global _MODULE
    if _MODULE is None:
        _MODULE = build_module()
    return _MODULE


def make_in_maps(inputs_c_e, W, b):
    x = np.asarray(inputs_c_e, np.float32)
    x_aug = np.concatenate([x, np.ones((B, L, 1), np.float32)], axis=2).astype(
        ml_dtypes.bfloat16
    )
    wt = build_wt_aug(W, b)
    spk = build_s_pack()
    return [
        {
            "x": np.ascontiguousarray(x_aug[c * B_LOC : (c + 1) * B_LOC]),
            "wt": wt,
            "spk": spk,
        }
        for c in range(NCORES)
    ]


def kernel(inputs_c_e, W, b):
    nc = _get_module()
    in_maps = make_in_maps(inputs_c_e, W, b)
    res = run_bass_kernel_spmd(nc, in_maps, core_ids=list(range(NCORES)))
    out = np.empty((5, B, L, D), np.float32)
    for c in range(NCORES):
        out[:, c * B_LOC : (c + 1) * B_LOC] = res.results[c]["out"]
    return out
